# revision 63
# baseline (speedup 1.0000x reference)
"""AdaptivePatchEmbed Trainium2 kernel.

Distribution: data-parallel over batch B=8 -> one sample per NeuronCore
(descriptors are identical per sample; small conv weights replicated).

Fast path (used when the descriptors have the 4x4-block structure that
the reference's _build_descs produces; a generic dma_gather path is the
fallback): everything is static HWDGE DMA -- no gpsimd ucode library, no
descriptor generation.
  - scale-0 tokens: ~8 fat static DRAM->DRAM copies in source order; the
    host applies the row permutation during output assembly.
  - conv inputs: the scale-1/scale-2 regions are t-complete 4x4 blocks, so
    they load as ~50 static [<=2 blocks, 16 rows, 1536B] DMAs into
    block-major SBUF tiles [rows, ch]; TensorE transposes them on-chip
    (via identity matmul) into channel-major XT tiles, and the conv
    matmuls read each (i,j)-shifted window directly with a 4-dim strided
    rhs access pattern [block, i', j', t] -- no data permutation at all.
    Output token order becomes (block, i', j', t); the host permutes the
    420 conv tokens back to descriptor order for free during assembly.
  - convs are einsums tok_out[d, n] = sum_{ij,c} W[(ij,c),d] * X[(ij,c),n]
    as 128x128-chunk matmuls accumulating in PSUM; conv2a output stays
    bf16 channel-major in SBUF and feeds conv2b directly.
Outputs per core: out0 [1408,768] f32 (scale-0 rows, source order) and
outT [768,420] f32 (tok1 ++ tok2 transposed, block order).
"""

import numpy as np
import ml_dtypes
from contextlib import ExitStack

# Problem constants (hardcoded; kernel.py must be self-contained).
B, H, W, T, D = 8, 32, 32, 4, 768
N0, N1, N2 = 1408, 336, 84
NPOS = H * W * T              # 4096 rows of D
KC = 24                       # K chunks of 128 over (i,j,c)=4*768
MC = 6                        # output-d chunks of 128
NTOK1 = N1                    # 336 conv1 tokens
NTOK2A = N2 * 4               # 336 conv2a output tokens
NSP = 84                      # spatial tokens per timestep (both convs)
NBLK = 21                     # spatial 4x4 blocks per conv region
NROW = NBLK * 64              # 1344 rows per conv region
NXT = 11                      # 128-row tiles per region (last half-full)
N_CORES = 8

_compiled = {}


def _flat_idx(y, x, t):
    return (y * W + x) * T + t


def _wrap_idxs(idx, pad_to):
    """int sequence -> int16 SBUF wrap layout [128, pad_to//16]."""
    idx = np.asarray(idx, np.int64)
    full = np.zeros(pad_to, np.int64)  # pad with valid idx 0 (junk, never read)
    full[: idx.size] = idx
    assert full.max() < 32768 and pad_to % 16 == 0
    wrapped = full.reshape(pad_to // 16, 16).T.astype(np.int16)  # [16, cols]
    return np.tile(wrapped, (8, 1))  # replicate across the 8 groups of 16


def _weight_mat(w):
    """w [D, D, 2, 2] -> [4D, D] bf16 with wm[(i*2+j)*D + c, d] = w[d,c,i,j].
    k-major layout gives the weight DMA 1536B descriptors, which share the
    SDMA packet round-robin fairly with concurrent SWDGE gather traffic."""
    wm = np.transpose(np.asarray(w, np.float32), (2, 3, 1, 0))  # [i, j, c, d]
    return np.ascontiguousarray(wm.reshape(4 * D, D)).astype(ml_dtypes.bfloat16)


def _bias_tile(b1, b2a, b2b):
    """[128, 18] f32: cols [g*6+m] = b_g[m*128+p]."""
    out = np.empty((128, 18), np.float32)
    for g, b in enumerate((b1, b2a, b2b)):
        out[:, g * 6 : (g + 1) * 6] = np.asarray(b, np.float32).reshape(MC, 128).T
    return out


def _runs_of(blocks):
    """blocks: iterable of (by, bx) 4-aligned. Returns ordered runs
    [(by, bxblk0, stepblk, L)] and the slot order (list of (by,bx))."""
    rows = {}
    for by, bx in blocks:
        rows.setdefault(by, []).append(bx // 4)
    runs, order = [], []
    for by in sorted(rows):
        bxs = sorted(rows[by])
        i = 0
        while i < len(bxs):
            j = i + 1
            step = 1
            if j < len(bxs):
                step = bxs[j] - bxs[i]
                while j + 1 < len(bxs) and bxs[j + 1] - bxs[j] == step:
                    j += 1
            runs.append((by, bxs[i], step, j - i))
            order += [(by, 4 * (bxs[i] + k * step)) for k in range(j - i)]
            i = j
    return runs, order


def _pieces_of(runs):
    """Split runs into <=2-block pieces aligned to even slots.
    Returns [(by, bxblk0, step, L, slot0)]."""
    pieces, c = [], 0
    for by, b0, s, L in runs:
        k = 0
        while k < L:
            if c % 2 == 0 and k + 1 < L:
                pieces.append((by, b0 + k * s, s, 2, c))
                k += 2
                c += 2
            else:
                pieces.append((by, b0 + k * s, s, 1, c))
                k += 1
                c += 1
    return pieces


def _tok0_blocks(desc0):
    """If desc0 is 88 4x4 raster blocks, return [(by, bx, t), ...] else None."""
    d0 = np.asarray(desc0, np.int64)
    if d0.shape != (N0, 3) or N0 % 16:
        return None
    blocks = d0.reshape(N0 // 16, 16, 3)
    by, bx, t = blocks[:, 0, 0], blocks[:, 0, 1], blocks[:, 0, 2]
    yy = by[:, None] + np.repeat(np.arange(4), 4)[None, :]
    xx = bx[:, None] + np.tile(np.arange(4), 4)[None, :]
    tt = np.broadcast_to(t[:, None], yy.shape)
    exp = np.stack([yy, xx, tt], axis=2)
    if not np.array_equal(exp, blocks) or yy.max() >= H or xx.max() >= W:
        return None
    return list(zip(by.tolist(), bx.tolist(), t.tolist()))


def _tok0_runs(tok0_blocks):
    """Returns (runs, pos_of) for the scale-0 fat copies, or (None, None)."""
    if tok0_blocks is None:
        return None, None
    byt = {}
    for by, bx, t in tok0_blocks:
        if by % 4 or bx % 4:
            return None, None
        byt.setdefault((by, bx), set()).add(t)
    if any(ts != set(range(T)) for ts in byt.values()):
        return None, None
    if len(byt) * 16 * T != N0:
        return None, None
    runs, _ = _runs_of(byt.keys())
    raw = []
    for by, b0, s, L in runs:
        for dy in range(4):
            for k in range(L):
                for dx in range(4):
                    for t in range(T):
                        raw.append(((by + dy) * W + (b0 + k * s) * 4 + dx) * T + t)
    raw = np.asarray(raw)
    pos_of = np.full(NPOS, -1, np.int64)
    pos_of[raw] = np.arange(N0)
    return runs, pos_of


def _spatial_lists(d1, d2):
    """Per-timestep spatial token lists (s1_yx [84,2], s2_yx [21,2]) or None."""
    d1 = np.asarray(d1, np.int64)
    d2 = np.asarray(d2, np.int64)
    if d1.shape != (N1, 3) or d2.shape != (N2, 3) or N1 % T or N2 % T:
        return None
    s1 = d1.reshape(T, N1 // T, 3)
    s2 = d2.reshape(T, N2 // T, 3)
    for s, win in ((s1, 2), (s2, 4)):
        if not np.array_equal(s[:, :, 2], np.broadcast_to(
                np.arange(T)[:, None], s.shape[:2])):
            return None
        if not all(np.array_equal(s[0, :, :2], s[t, :, :2]) for t in range(T)):
            return None
        yx = s[0, :, :2]
        if (yx % 2).any() or yx[:, 0].max() + win > H or yx[:, 1].max() + win > W:
            return None
    return s1[0, :, :2], s2[0, :, :2]


def _conv_layout(s1_yx, s2_yx):
    """Block structure for the static conv loads. Returns
    (pieces1, pieces2, perm1, perm2) or None if structure doesn't hold."""
    # s1: group the 84 2x2-token positions into 21 4-aligned 4x4 blocks
    blocks1 = {}
    for s, (y, x) in enumerate(s1_yx):
        by, bx = (y // 4) * 4, (x // 4) * 4
        if y - by not in (0, 2) or x - bx not in (0, 2):
            return None
        blocks1.setdefault((by, bx), {})[((y - by) // 2, (x - bx) // 2)] = s
    if len(blocks1) != NBLK or any(len(v) != 4 for v in blocks1.values()):
        return None
    # s2: the 21 4x4 windows must be 4-aligned blocks
    blocks2 = {}
    for k, (by, bx) in enumerate(s2_yx):
        if by % 4 or bx % 4 or (by, bx) in blocks2:
            return None
        blocks2[(by, bx)] = k
    if len(blocks2) != NBLK:
        return None

    runs1, order1 = _runs_of(blocks1.keys())
    runs2, order2 = _runs_of(blocks2.keys())
    if len(order1) != NBLK or len(order2) != NBLK:
        return None
    slot1 = {b: i for i, b in enumerate(order1)}
    slot2 = {b: i for i, b in enumerate(order2)}

    # conv1 out col for desc1 row (t*84 + s): j'*168 + t*42 + slot*2 + i'
    perm1 = np.empty(N1, np.int64)
    for (by, bx), toks in blocks1.items():
        sl = slot1[(by, bx)]
        for (ip, jp), s in toks.items():
            for t in range(T):
                perm1[t * (N1 // T) + s] = jp * 168 + t * 42 + sl * 2 + ip
    # conv2b out col for desc2 row (t*21 + k): t*21 + slot2
    perm2 = np.empty(N2, np.int64)
    for (by, bx), k in blocks2.items():
        sl = slot2[(by, bx)]
        for t in range(T):
            perm2[t * NBLK + k] = t * NBLK + sl
    return runs1, runs2, perm1, perm2


def _super_idxs(s1_yx, s2_yx):
    """Super-row gather indices (base viewed as [1024, 4*768]): one element
    covers (y, x, all t). (i,j)-major groups. Returns (idx1 [336], idx2 [336])."""
    def srow(y, x):
        return y * W + x

    idx1 = np.concatenate([
        np.asarray([srow(y + i, x + j) for y, x in s1_yx])
        for i in range(2) for j in range(2)])
    # conv2a spatial order (blk, h, w) inside each (i,j)-group
    idx2 = np.concatenate([
        np.asarray([srow(by + 2 * h + i, bx + 2 * w + j)
                    for by, bx in s2_yx for h in range(2) for w in range(2)])
        for i in range(2) for j in range(2)])
    return idx1, idx2


def _build_bass(cfg):
    import concourse.bacc as bacc
    import concourse.tile as tile
    from concourse import mybir

    fast = cfg["fast"]
    nc = bacc.Bacc("TRN2", target_bir_lowering=False, debug=False,
                   num_devices=N_CORES, num_swdge_queues=1,
                   dynamic_dma_scratch_size=32768)
    dt = mybir.dt

    base_f32 = nc.dram_tensor("base_f32", (NPOS, D), dt.float32, kind="ExternalInput")
    base_bf16 = nc.dram_tensor("base_bf16", (NPOS, D), dt.bfloat16, kind="ExternalInput")
    w1m = nc.dram_tensor("w1m", (4 * D, D), dt.bfloat16, kind="ExternalInput")
    w2am = nc.dram_tensor("w2am", (4 * D, D), dt.bfloat16, kind="ExternalInput")
    w2bm = nc.dram_tensor("w2bm", (4 * D, D), dt.bfloat16, kind="ExternalInput")
    biases = nc.dram_tensor("biases", (128, 18), dt.float32, kind="ExternalInput")
    n_idx_cols = 48 if fast else 280
    idxs = nc.dram_tensor("idxs", (128, n_idx_cols), dt.int16, kind="ExternalInput")
    out0 = nc.dram_tensor("out0", (N0, D), dt.float32, kind="ExternalOutput")
    outT = nc.dram_tensor("outT", (D, NTOK1 + N2), dt.float32, kind="ExternalOutput")

    with ExitStack() as ctx:
        tc = ctx.enter_context(tile.TileContext(nc))
        consts = ctx.enter_context(tc.tile_pool(name="consts", bufs=1))
        wpool = ctx.enter_context(tc.tile_pool(name="wpool", bufs=1))
        xpool = ctx.enter_context(tc.tile_pool(name="xpool", bufs=1))
        gpool = ctx.enter_context(tc.tile_pool(name="gpool", bufs=1))
        opool = ctx.enter_context(tc.tile_pool(name="opool", bufs=1))
        psum = ctx.enter_context(tc.tile_pool(name="psum", bufs=4, space="PSUM"))

        # idx upload first on the sync ring: it gates the gathers
        idx_s = consts.tile([128, n_idx_cols], dt.int16)
        nc.sync.dma_start(idx_s[:], idxs.ap()[:])
        bias_s = consts.tile([128, 18], dt.float32)
        nc.scalar.dma_start(bias_s[:], biases.ap()[:])

        if fast:
            # Super-row gathers: 384 idxs of 6.1KB elements (1 x-col x 4t
            # x 768ch contiguous); output [128, 24, 384] with middle dim
            # e = t*6 + c6. Tiny Q7 descriptor-gen (~4us).
            in_v = base_bf16.ap().rearrange("(s a) d -> s (a d)", a=4)
            g1s = gpool.tile([128, 24, 384], dt.bfloat16, tag="g1")
            nc.gpsimd.dma_gather(
                g1s[:], in_v, idx_s[:, 0:24],
                num_idxs=384, num_idxs_reg=384, elem_size=4 * D, transpose=True,
                single_packet=False,
            )
            g2s = gpool.tile([128, 24, 384], dt.bfloat16, tag="g2a")
            nc.gpsimd.dma_gather(
                g2s[:], in_v, idx_s[:, 24:48],
                num_idxs=384, num_idxs_reg=384, elem_size=4 * D, transpose=True,
                single_packet=False,
            )

            def rhs_conv(g, ij, c6, _ntok):
                # cols (t, n): e = t*6 + c6 ; n-group at ij*84
                return g[:, c6 : c6 + 19 : 6, ij * NSP : (ij + 1) * NSP]

            r1key, r2key = g1s, g2s
        else:
            ghalves = []
            for gi in range(4):
                gh = gpool.tile([128, MC, 768], dt.bfloat16, tag=f"gh{gi}")
                nc.gpsimd.dma_gather(
                    gh[:], base_bf16.ap()[:], idx_s[:, gi * 48 : gi * 48 + 48],
                    num_idxs=768, num_idxs_reg=768, elem_size=D, transpose=True,
                    single_packet=False,
                )
                ghalves.append(gh)

            def rhs_conv(gi_base, ij, c6, ntok):
                half = ghalves[gi_base + ij // 2]
                ijl = ij % 2
                return half[:, c6, ijl * ntok : (ijl + 1) * ntok]

            r1key, r2key = 0, 2

        # Weights -> SBUF [128, KC, D] (partition-major DRAM layout).
        # w2b shares w1's slot (loads after conv1 releases it).
        wts = []
        for wdram, nm, tg, eng in (
                (w1m, "w1", "wA", nc.sync), (w2am, "w2a", "wB", nc.scalar),
                (w2bm, "w2b", "wA", nc.sync)):
            wt = wpool.tile([128, KC, D], dt.bfloat16, tag=tg, name=nm)
            eng.dma_start(wt[:], wdram.ap().rearrange("(k p) d -> p k d", p=128))
            wts.append(wt)
        w1s, w2as, w2bs = wts

        # conv1
        out1 = opool.tile([128, MC, NTOK1], dt.float32, tag="out1")
        for m in range(MC):
            ps = psum.tile([128, NTOK1], dt.float32, tag="ps")
            for kc in range(KC):
                ij, c6 = divmod(kc, MC)
                nc.tensor.matmul(
                    ps[:],
                    w1s[:, kc, m * 128 : (m + 1) * 128],
                    rhs_conv(r1key, ij, c6, NTOK1),
                    start=(kc == 0), stop=(kc == KC - 1),
                )
            nc.vector.tensor_scalar_add(out1[:, m, :], ps[:], bias_s[:, m : m + 1])

        # conv2a (bf16 output feeds conv2b)
        out2a = opool.tile([128, MC, NTOK2A], dt.bfloat16, tag="out2a")
        for m in range(MC):
            ps = psum.tile([128, NTOK2A], dt.float32, tag="ps")
            for kc in range(KC):
                ij, c6 = divmod(kc, MC)
                nc.tensor.matmul(
                    ps[:],
                    w2as[:, kc, m * 128 : (m + 1) * 128],
                    rhs_conv(r2key, ij, c6, NTOK2A),
                    start=(kc == 0), stop=(kc == KC - 1),
                )
            nc.vector.tensor_scalar_add(out2a[:, m, :], ps[:], bias_s[:, 6 + m : 7 + m])

        # conv2b
        out2b = opool.tile([128, MC, N2], dt.float32, tag="out2b")
        for m in range(MC):
            ps = psum.tile([128, N2], dt.float32, tag="ps")
            for kc in range(KC):
                ij, c6 = divmod(kc, MC)
                if fast:
                    # out2a cols (t, blk, h, w): select (h,w)=(i,j)
                    rhs2b = out2a[:, c6, ij : ij + 4 * (N2 - 1) + 1 : 4]
                else:
                    rhs2b = out2a[:, c6, ij * N2 : (ij + 1) * N2]
                nc.tensor.matmul(
                    ps[:],
                    w2bs[:, kc, m * 128 : (m + 1) * 128],
                    rhs2b,
                    start=(kc == 0), stop=(kc == KC - 1),
                )
            nc.vector.tensor_scalar_add(out2b[:, m, :], ps[:], bias_s[:, 12 + m : 13 + m])

        # scale-0: fat static DRAM->DRAM copies in source order
        if cfg["tok0_runs"] is not None:
            base_y = base_f32.ap().rearrange("(y bx rr) d -> y bx rr d",
                                             bx=W // 4, rr=4 * T)
            off = 0
            for k, (by, b0, s, L) in enumerate(cfg["tok0_runs"]):
                nrows = 4 * L * 4 * T
                eng = nc.scalar if k % 2 == 0 else nc.sync
                eng.dma_start(
                    out0.ap()[off : off + nrows, :],
                    base_y[by : by + 4, b0 : b0 + (L - 1) * s + 1 : s, :, :],
                )
                off += nrows
        else:
            g0 = gpool.tile([128, N0 // 128, D], dt.float32, tag="g0")
            nc.gpsimd.dma_gather(
                g0[:], base_f32.ap()[:], idx_s[:, 192:280],
                num_idxs=N0, num_idxs_reg=N0, elem_size=D, single_packet=False,
            )
            nc.sync.dma_start(
                out0.ap().rearrange("(g p) d -> p g d", p=128), g0[:]
            )

        # outputs: outT [768, 420] viewed [6, 128, 420]
        outT_v = outT.ap().rearrange("(m p) n -> p m n", p=128)
        nc.sync.dma_start(outT_v[:, :, 0:NTOK1], out1[:])
        nc.sync.dma_start(outT_v[:, :, NTOK1 : NTOK1 + N2], out2b[:])

    nc.finalize()
    return nc


def _prep_shared(desc0, desc1, desc2, w1, b1, w2a, b2a, w2b, b2b):
    """Host-side shared (core-independent) input prep. Returns (shared,
    positions, cfg, perm0, colperm)."""
    d0 = np.asarray(desc0, np.int64)
    d1 = np.asarray(desc1, np.int64)
    d2 = np.asarray(desc2, np.int64)

    tok0_blocks = _tok0_blocks(d0)
    tok0_runs, pos_of = _tok0_runs(tok0_blocks)
    sp = _spatial_lists(d1, d2)
    fast = sp is not None and tok0_runs is not None

    shared = {
        "w1m": _weight_mat(w1),
        "w2am": _weight_mat(w2a),
        "w2bm": _weight_mat(w2b),
        "biases": _bias_tile(b1, b2a, b2b),
    }

    perm0 = None
    colperm = None
    if fast:
        d0flat = _flat_idx(d0[:, 0], d0[:, 1], d0[:, 2])
        perm0 = pos_of[d0flat]
        assert perm0.min() >= 0
        idx1, idx2 = _super_idxs(*sp)
        shared["idxs"] = np.concatenate(
            [_wrap_idxs(idx1, 384), _wrap_idxs(idx2, 384)], axis=1)
        cfg = {"fast": True, "tok0_runs": tok0_runs}
    else:
        # generic fallback: per-row transpose gathers + tok0 gather
        idx1_groups = [
            _flat_idx(d1[:, 0] + i, d1[:, 1] + j, d1[:, 2])
            for i in range(2) for j in range(2)
        ]
        hh, ww = np.arange(2), np.arange(2)
        idx2_groups = [
            _flat_idx(
                (d2[:, 0][None, None, :] + 2 * hh[:, None, None] + i),
                (d2[:, 1][None, None, :] + 2 * ww[None, :, None] + j),
                np.broadcast_to(d2[:, 2][None, None, :], (2, 2, N2)),
            ).ravel()
            for i in range(2) for j in range(2)
        ]
        halves = [np.concatenate(idx1_groups[0:2]), np.concatenate(idx1_groups[2:4]),
                  np.concatenate(idx2_groups[0:2]), np.concatenate(idx2_groups[2:4])]
        parts = [_wrap_idxs(h, 768) for h in halves]
        idx0 = _flat_idx(d0[:, 0], d0[:, 1], d0[:, 2])
        parts.append(_wrap_idxs(idx0, 1408))
        shared["idxs"] = np.concatenate(parts, axis=1)
        cfg = {"fast": False, "tok0_runs": None}

    def _pos(desc, size):
        n = desc.shape[0]
        return np.concatenate(
            [desc[:, :2].astype(np.int32),
             np.full((n, 1), size, np.int32),
             desc[:, 2:3].astype(np.int32)], axis=1)

    positions = np.concatenate(
        [_pos(np.asarray(desc0, np.int32), 1),
         _pos(np.asarray(desc1, np.int32), 2),
         _pos(np.asarray(desc2, np.int32), 4)], axis=0)
    return shared, positions, cfg, perm0, colperm


def kernel(base_patch_embeddings, desc0, desc1, desc2,
           w1, b1, w2a, b2a, w2b, b2b):
    from concourse.bass_utils import run_bass_kernel_spmd

    base = np.asarray(base_patch_embeddings, np.float32)
    assert base.shape == (B, H, W, T, D)

    shared, positions, cfg, perm0, colperm = _prep_shared(
        desc0, desc1, desc2, w1, b1, w2a, b2a, w2b, b2b)

    key = repr(sorted((k, repr(v)) for k, v in cfg.items()))
    if key not in _compiled:
        _compiled[key] = _build_bass(cfg)
    nc = _compiled[key]

    in_maps = []
    for b in range(B):
        sample = np.ascontiguousarray(base[b].reshape(NPOS, D))
        m = dict(shared)
        m["base_f32"] = sample
        m["base_bf16"] = sample.astype(ml_dtypes.bfloat16)
        in_maps.append(m)

    res = run_bass_kernel_spmd(nc, in_maps, core_ids=list(range(N_CORES)))

    tokens = np.empty((B, N0 + N1 + N2, D), np.float32)
    for b in range(B):
        o0 = res.results[b]["out0"]
        oT = res.results[b]["outT"].T                   # [420, 768]
        tokens[b, :N0] = o0[perm0] if perm0 is not None else o0
        tokens[b, N0:] = oT[colperm] if colperm is not None else oT
    return tokens, positions


# revision 67
# speedup vs baseline: 1.1977x; 1.1977x over previous
"""AdaptivePatchEmbed Trainium2 kernel.

Distribution: data-parallel over batch B=8 -> one sample per NeuronCore
(descriptors are identical per sample; small conv weights replicated).

Fast path (used when the descriptors have the 4x4-block structure that
the reference's _build_descs produces; a generic dma_gather path is the
fallback): everything is static HWDGE DMA -- no gpsimd ucode library, no
descriptor generation.
  - scale-0 tokens: ~8 fat static DRAM->DRAM copies in source order; the
    host applies the row permutation during output assembly.
  - conv inputs: the scale-1/scale-2 regions are t-complete 4x4 blocks, so
    they load as ~50 static [<=2 blocks, 16 rows, 1536B] DMAs into
    block-major SBUF tiles [rows, ch]; TensorE transposes them on-chip
    (via identity matmul) into channel-major XT tiles, and the conv
    matmuls read each (i,j)-shifted window directly with a 4-dim strided
    rhs access pattern [block, i', j', t] -- no data permutation at all.
    Output token order becomes (block, i', j', t); the host permutes the
    420 conv tokens back to descriptor order for free during assembly.
  - convs are einsums tok_out[d, n] = sum_{ij,c} W[(ij,c),d] * X[(ij,c),n]
    as 128x128-chunk matmuls accumulating in PSUM; conv2a output stays
    bf16 channel-major in SBUF and feeds conv2b directly.
Outputs per core: out0 [1408,768] f32 (scale-0 rows, source order) and
outT [768,420] f32 (tok1 ++ tok2 transposed, block order).
"""

import numpy as np
import ml_dtypes
from contextlib import ExitStack

# Problem constants (hardcoded; kernel.py must be self-contained).
B, H, W, T, D = 8, 32, 32, 4, 768
N0, N1, N2 = 1408, 336, 84
NPOS = H * W * T              # 4096 rows of D
KC = 24                       # K chunks of 128 over (i,j,c)=4*768
MC = 6                        # output-d chunks of 128
NTOK1 = N1                    # 336 conv1 tokens
NTOK2A = N2 * 4               # 336 conv2a output tokens
NSP = 84                      # spatial tokens per timestep (both convs)
NBLK = 21                     # spatial 4x4 blocks per conv region
NROW = NBLK * 64              # 1344 rows per conv region
NXT = 11                      # 128-row tiles per region (last half-full)
N_CORES = 8

_compiled = {}


def _flat_idx(y, x, t):
    return (y * W + x) * T + t


def _wrap_idxs(idx, pad_to):
    """int sequence -> int16 SBUF wrap layout [128, pad_to//16]."""
    idx = np.asarray(idx, np.int64)
    full = np.zeros(pad_to, np.int64)  # pad with valid idx 0 (junk, never read)
    full[: idx.size] = idx
    assert full.max() < 32768 and pad_to % 16 == 0
    wrapped = full.reshape(pad_to // 16, 16).T.astype(np.int16)  # [16, cols]
    return np.tile(wrapped, (8, 1))  # replicate across the 8 groups of 16


def _weight_mat(w):
    """w [D, D, 2, 2] -> [4D, D] bf16 with wm[(i*2+j)*D + c, d] = w[d,c,i,j].
    k-major layout gives the weight DMA 1536B descriptors, which share the
    SDMA packet round-robin fairly with concurrent SWDGE gather traffic."""
    wm = np.transpose(np.asarray(w, np.float32), (2, 3, 1, 0))  # [i, j, c, d]
    return np.ascontiguousarray(wm.reshape(4 * D, D)).astype(ml_dtypes.bfloat16)


def _bias_tile(b1, b2a, b2b):
    """[128, 18] f32: cols [g*6+m] = b_g[m*128+p]."""
    out = np.empty((128, 18), np.float32)
    for g, b in enumerate((b1, b2a, b2b)):
        out[:, g * 6 : (g + 1) * 6] = np.asarray(b, np.float32).reshape(MC, 128).T
    return out


def _runs_of(blocks):
    """blocks: iterable of (by, bx) 4-aligned. Returns ordered runs
    [(by, bxblk0, stepblk, L)] and the slot order (list of (by,bx))."""
    rows = {}
    for by, bx in blocks:
        rows.setdefault(by, []).append(bx // 4)
    runs, order = [], []
    for by in sorted(rows):
        bxs = sorted(rows[by])
        i = 0
        while i < len(bxs):
            j = i + 1
            step = 1
            if j < len(bxs):
                step = bxs[j] - bxs[i]
                while j + 1 < len(bxs) and bxs[j + 1] - bxs[j] == step:
                    j += 1
            runs.append((by, bxs[i], step, j - i))
            order += [(by, 4 * (bxs[i] + k * step)) for k in range(j - i)]
            i = j
    return runs, order


def _pieces_of(runs):
    """Split runs into <=2-block pieces aligned to even slots.
    Returns [(by, bxblk0, step, L, slot0)]."""
    pieces, c = [], 0
    for by, b0, s, L in runs:
        k = 0
        while k < L:
            if c % 2 == 0 and k + 1 < L:
                pieces.append((by, b0 + k * s, s, 2, c))
                k += 2
                c += 2
            else:
                pieces.append((by, b0 + k * s, s, 1, c))
                k += 1
                c += 1
    return pieces


def _tok0_blocks(desc0):
    """If desc0 is 88 4x4 raster blocks, return [(by, bx, t), ...] else None."""
    d0 = np.asarray(desc0, np.int64)
    if d0.shape != (N0, 3) or N0 % 16:
        return None
    blocks = d0.reshape(N0 // 16, 16, 3)
    by, bx, t = blocks[:, 0, 0], blocks[:, 0, 1], blocks[:, 0, 2]
    yy = by[:, None] + np.repeat(np.arange(4), 4)[None, :]
    xx = bx[:, None] + np.tile(np.arange(4), 4)[None, :]
    tt = np.broadcast_to(t[:, None], yy.shape)
    exp = np.stack([yy, xx, tt], axis=2)
    if not np.array_equal(exp, blocks) or yy.max() >= H or xx.max() >= W:
        return None
    return list(zip(by.tolist(), bx.tolist(), t.tolist()))


def _tok0_runs(tok0_blocks):
    """Returns (runs, pos_of) for the scale-0 fat copies, or (None, None)."""
    if tok0_blocks is None:
        return None, None
    byt = {}
    for by, bx, t in tok0_blocks:
        if by % 4 or bx % 4:
            return None, None
        byt.setdefault((by, bx), set()).add(t)
    if any(ts != set(range(T)) for ts in byt.values()):
        return None, None
    if len(byt) * 16 * T != N0:
        return None, None
    runs, _ = _runs_of(byt.keys())
    raw = []
    for by, b0, s, L in runs:
        for dy in range(4):
            for k in range(L):
                for dx in range(4):
                    for t in range(T):
                        raw.append(((by + dy) * W + (b0 + k * s) * 4 + dx) * T + t)
    raw = np.asarray(raw)
    pos_of = np.full(NPOS, -1, np.int64)
    pos_of[raw] = np.arange(N0)
    return runs, pos_of


def _spatial_lists(d1, d2):
    """Per-timestep spatial token lists (s1_yx [84,2], s2_yx [21,2]) or None."""
    d1 = np.asarray(d1, np.int64)
    d2 = np.asarray(d2, np.int64)
    if d1.shape != (N1, 3) or d2.shape != (N2, 3) or N1 % T or N2 % T:
        return None
    s1 = d1.reshape(T, N1 // T, 3)
    s2 = d2.reshape(T, N2 // T, 3)
    for s, win in ((s1, 2), (s2, 4)):
        if not np.array_equal(s[:, :, 2], np.broadcast_to(
                np.arange(T)[:, None], s.shape[:2])):
            return None
        if not all(np.array_equal(s[0, :, :2], s[t, :, :2]) for t in range(T)):
            return None
        yx = s[0, :, :2]
        if (yx % 2).any() or yx[:, 0].max() + win > H or yx[:, 1].max() + win > W:
            return None
    return s1[0, :, :2], s2[0, :, :2]


def _conv_layout(s1_yx, s2_yx):
    """Block structure for the static conv loads. Returns
    (pieces1, pieces2, perm1, perm2) or None if structure doesn't hold."""
    # s1: group the 84 2x2-token positions into 21 4-aligned 4x4 blocks
    blocks1 = {}
    for s, (y, x) in enumerate(s1_yx):
        by, bx = (y // 4) * 4, (x // 4) * 4
        if y - by not in (0, 2) or x - bx not in (0, 2):
            return None
        blocks1.setdefault((by, bx), {})[((y - by) // 2, (x - bx) // 2)] = s
    if len(blocks1) != NBLK or any(len(v) != 4 for v in blocks1.values()):
        return None
    # s2: the 21 4x4 windows must be 4-aligned blocks
    blocks2 = {}
    for k, (by, bx) in enumerate(s2_yx):
        if by % 4 or bx % 4 or (by, bx) in blocks2:
            return None
        blocks2[(by, bx)] = k
    if len(blocks2) != NBLK:
        return None

    runs1, order1 = _runs_of(blocks1.keys())
    runs2, order2 = _runs_of(blocks2.keys())
    if len(order1) != NBLK or len(order2) != NBLK:
        return None
    slot1 = {b: i for i, b in enumerate(order1)}
    slot2 = {b: i for i, b in enumerate(order2)}

    # conv1 out col for desc1 row (t*84 + s): j'*168 + t*42 + slot*2 + i'
    perm1 = np.empty(N1, np.int64)
    for (by, bx), toks in blocks1.items():
        sl = slot1[(by, bx)]
        for (ip, jp), s in toks.items():
            for t in range(T):
                perm1[t * (N1 // T) + s] = jp * 168 + t * 42 + sl * 2 + ip
    # conv2b out col for desc2 row (t*21 + k): t*21 + slot2
    perm2 = np.empty(N2, np.int64)
    for (by, bx), k in blocks2.items():
        sl = slot2[(by, bx)]
        for t in range(T):
            perm2[t * NBLK + k] = t * NBLK + sl
    return runs1, runs2, perm1, perm2


def _super_idxs(s1_yx, s2_yx):
    """Super-row gather indices (base viewed as [1024, 4*768]): one element
    covers (y, x, all t). (i,j)-major groups. Returns (idx1 [336], idx2 [336])."""
    def srow(y, x):
        return y * W + x

    idx1 = np.concatenate([
        np.asarray([srow(y + i, x + j) for y, x in s1_yx])
        for i in range(2) for j in range(2)])
    # conv2a spatial order (blk, h, w) inside each (i,j)-group
    idx2 = np.concatenate([
        np.asarray([srow(by + 2 * h + i, bx + 2 * w + j)
                    for by, bx in s2_yx for h in range(2) for w in range(2)])
        for i in range(2) for j in range(2)])
    return idx1, idx2


def _build_bass(cfg):
    import concourse.bacc as bacc
    import concourse.tile as tile
    from concourse import mybir

    fast = cfg["fast"]
    nc = bacc.Bacc("TRN2", target_bir_lowering=False, debug=False,
                   num_devices=N_CORES, num_swdge_queues=1,
                   dynamic_dma_scratch_size=24576)
    dt = mybir.dt

    base_f32 = nc.dram_tensor("base_f32", (NPOS, D), dt.float32, kind="ExternalInput")
    base_bf16 = nc.dram_tensor("base_bf16", (NPOS, D), dt.bfloat16, kind="ExternalInput")
    w1m = nc.dram_tensor("w1m", (4 * D, D), dt.bfloat16, kind="ExternalInput")
    w2am = nc.dram_tensor("w2am", (4 * D, D), dt.bfloat16, kind="ExternalInput")
    w2bm = nc.dram_tensor("w2bm", (4 * D, D), dt.bfloat16, kind="ExternalInput")
    biases = nc.dram_tensor("biases", (128, 18), dt.float32, kind="ExternalInput")
    n_idx_cols = 136 if fast else 280
    idxs = nc.dram_tensor("idxs", (128, n_idx_cols), dt.int16, kind="ExternalInput")
    out0 = nc.dram_tensor("out0", (N0, D), dt.float32, kind="ExternalOutput")
    outT = nc.dram_tensor("outT", (D, NTOK1 + N2), dt.float32, kind="ExternalOutput")

    with ExitStack() as ctx:
        tc = ctx.enter_context(tile.TileContext(nc))
        consts = ctx.enter_context(tc.tile_pool(name="consts", bufs=1))
        wpool = ctx.enter_context(tc.tile_pool(name="wpool", bufs=1))
        xpool = ctx.enter_context(tc.tile_pool(name="xpool", bufs=1))
        gpool = ctx.enter_context(tc.tile_pool(name="gpool", bufs=1))
        opool = ctx.enter_context(tc.tile_pool(name="opool", bufs=1))
        psum = ctx.enter_context(tc.tile_pool(name="psum", bufs=4, space="PSUM"))

        # idx upload first on the sync ring: it gates the gathers
        idx_s = consts.tile([128, n_idx_cols], dt.int16)
        nc.sync.dma_start(idx_s[:], idxs.ap()[:])
        bias_s = consts.tile([128, 18], dt.float32)
        nc.scalar.dma_start(bias_s[:], biases.ap()[:])

        if fast:
            # Super-row gathers: 384 idxs of 6.1KB elements (1 x-col x 4t
            # x 768ch contiguous); output [128, 24, 384] with middle dim
            # e = t*6 + c6. Tiny Q7 descriptor-gen (~4us).
            in_v = base_bf16.ap().rearrange("(s a) d -> s (a d)", a=4)
            g1s = gpool.tile([128, 24, 384], dt.bfloat16, tag="g1")
            nc.gpsimd.dma_gather(
                g1s[:], in_v, idx_s[:, 0:24],
                num_idxs=384, num_idxs_reg=384, elem_size=4 * D, transpose=True,
                single_packet=False,
            )
            g2s = gpool.tile([128, 24, 384], dt.bfloat16, tag="g2a")
            nc.gpsimd.dma_gather(
                g2s[:], in_v, idx_s[:, 24:48],
                num_idxs=384, num_idxs_reg=384, elem_size=4 * D, transpose=True,
                single_packet=False,
            )

            def rhs_conv(g, ij, c6, _ntok):
                # cols (t, n): e = t*6 + c6 ; n-group at ij*84
                return g[:, c6 : c6 + 19 : 6, ij * NSP : (ij + 1) * NSP]

            r1key, r2key = g1s, g2s
        else:
            ghalves = []
            for gi in range(4):
                gh = gpool.tile([128, MC, 768], dt.bfloat16, tag=f"gh{gi}")
                nc.gpsimd.dma_gather(
                    gh[:], base_bf16.ap()[:], idx_s[:, gi * 48 : gi * 48 + 48],
                    num_idxs=768, num_idxs_reg=768, elem_size=D, transpose=True,
                    single_packet=False,
                )
                ghalves.append(gh)

            def rhs_conv(gi_base, ij, c6, ntok):
                half = ghalves[gi_base + ij // 2]
                ijl = ij % 2
                return half[:, c6, ijl * ntok : (ijl + 1) * ntok]

            r1key, r2key = 0, 2

        # Weights -> SBUF [128, KC, D] (partition-major DRAM layout).
        # w2b shares w1's slot (loads after conv1 releases it).
        wts = []
        for wdram, nm, tg, eng in (
                (w1m, "w1", "wA", nc.sync), (w2am, "w2a", "wB", nc.scalar),
                (w2bm, "w2b", "wA", nc.sync)):
            wt = wpool.tile([128, KC, D], dt.bfloat16, tag=tg, name=nm)
            eng.dma_start(wt[:], wdram.ap().rearrange("(k p) d -> p k d", p=128))
            wts.append(wt)
        w1s, w2as, w2bs = wts

        outT_v1 = outT.ap().rearrange("(m p) n -> p m n", p=128)

        # conv1
        out1 = opool.tile([128, MC, NTOK1], dt.float32, tag="out1")
        for m in range(MC):
            ps = psum.tile([128, NTOK1], dt.float32, tag="ps")
            for kc in range(KC):
                ij, c6 = divmod(kc, MC)
                nc.tensor.matmul(
                    ps[:],
                    w1s[:, kc, m * 128 : (m + 1) * 128],
                    rhs_conv(r1key, ij, c6, NTOK1),
                    start=(kc == 0), stop=(kc == KC - 1),
                )
            nc.vector.tensor_scalar_add(out1[:, m, :], ps[:], bias_s[:, m : m + 1])
            nc.sync.dma_start(outT_v1[:, m, 0:NTOK1], out1[:, m, :])

        # conv2a (bf16 output feeds conv2b)
        out2a = opool.tile([128, MC, NTOK2A], dt.bfloat16, tag="out2a")
        for m in range(MC):
            ps = psum.tile([128, NTOK2A], dt.float32, tag="ps")
            for kc in range(KC):
                ij, c6 = divmod(kc, MC)
                nc.tensor.matmul(
                    ps[:],
                    w2as[:, kc, m * 128 : (m + 1) * 128],
                    rhs_conv(r2key, ij, c6, NTOK2A),
                    start=(kc == 0), stop=(kc == KC - 1),
                )
            nc.vector.tensor_scalar_add(out2a[:, m, :], ps[:], bias_s[:, 6 + m : 7 + m])

        # conv2b
        out2b = opool.tile([128, MC, N2], dt.float32, tag="out2b")
        for m in range(MC):
            ps = psum.tile([128, N2], dt.float32, tag="ps")
            for kc in range(KC):
                ij, c6 = divmod(kc, MC)
                if fast:
                    # out2a cols (t, blk, h, w): select (h,w)=(i,j)
                    rhs2b = out2a[:, c6, ij : ij + 4 * (N2 - 1) + 1 : 4]
                else:
                    rhs2b = out2a[:, c6, ij * N2 : (ij + 1) * N2]
                nc.tensor.matmul(
                    ps[:],
                    w2bs[:, kc, m * 128 : (m + 1) * 128],
                    rhs2b,
                    start=(kc == 0), stop=(kc == KC - 1),
                )
            nc.vector.tensor_scalar_add(out2b[:, m, :], ps[:], bias_s[:, 12 + m : 13 + m])
            nc.sync.dma_start(outT_v1[:, m, NTOK1 : NTOK1 + N2], out2b[:, m, :])

        # scale-0: quartered SWDGE row gathers (queued after the conv
        # gathers on the Q7 path) + fat SBUF->DRAM writes; avoids the slow
        # DRAM->DRAM path entirely.
        if cfg["fast"]:
            off_rows, off_cols = 0, 48
            for q, nq in enumerate((384, 384, 384, 256)):
                g0 = gpool.tile([128, 3, D], dt.float32, tag="g0", bufs=2,
                                name=f"g0_{q}")
                gq = nq // 128
                nc.gpsimd.dma_gather(
                    g0[:, 0:gq, :], base_f32.ap()[:],
                    idx_s[:, off_cols : off_cols + nq // 16],
                    num_idxs=nq, num_idxs_reg=nq, elem_size=D,
                    single_packet=False,
                )
                eng = nc.sync if q % 2 == 0 else nc.scalar
                eng.dma_start(
                    out0.ap()[off_rows : off_rows + nq, :]
                    .rearrange("(g p) d -> p g d", p=128),
                    g0[:, 0:gq, :],
                )
                off_rows += nq
                off_cols += nq // 16
        elif cfg["tok0_runs"] is not None:
            base_y = base_f32.ap().rearrange("(y bx rr) d -> y bx rr d",
                                             bx=W // 4, rr=4 * T)
            off = 0
            for k, (by, b0, s, L) in enumerate(cfg["tok0_runs"]):
                nrows = 4 * L * 4 * T
                eng = nc.scalar if k % 2 == 0 else nc.sync
                eng.dma_start(
                    out0.ap()[off : off + nrows, :],
                    base_y[by : by + 4, b0 : b0 + (L - 1) * s + 1 : s, :, :],
                )
                off += nrows
        else:
            g0 = gpool.tile([128, N0 // 128, D], dt.float32, tag="g0")
            nc.gpsimd.dma_gather(
                g0[:], base_f32.ap()[:], idx_s[:, 192:280],
                num_idxs=N0, num_idxs_reg=N0, elem_size=D, single_packet=False,
            )
            nc.sync.dma_start(
                out0.ap().rearrange("(g p) d -> p g d", p=128), g0[:]
            )


    nc.finalize()
    return nc


def _prep_shared(desc0, desc1, desc2, w1, b1, w2a, b2a, w2b, b2b):
    """Host-side shared (core-independent) input prep. Returns (shared,
    positions, cfg, perm0, colperm)."""
    d0 = np.asarray(desc0, np.int64)
    d1 = np.asarray(desc1, np.int64)
    d2 = np.asarray(desc2, np.int64)

    tok0_blocks = _tok0_blocks(d0)
    tok0_runs, pos_of = _tok0_runs(tok0_blocks)
    sp = _spatial_lists(d1, d2)
    fast = sp is not None

    shared = {
        "w1m": _weight_mat(w1),
        "w2am": _weight_mat(w2a),
        "w2bm": _weight_mat(w2b),
        "biases": _bias_tile(b1, b2a, b2b),
    }

    perm0 = None
    colperm = None
    if fast:
        idx1, idx2 = _super_idxs(*sp)
        idx0 = _flat_idx(d0[:, 0], d0[:, 1], d0[:, 2])
        bounds = [0, 384, 768, 1152, 1408]
        quarters = [_wrap_idxs(idx0[bounds[q] : bounds[q + 1]],
                               bounds[q + 1] - bounds[q]) for q in range(4)]
        shared["idxs"] = np.concatenate(
            [_wrap_idxs(idx1, 384), _wrap_idxs(idx2, 384)] + quarters, axis=1)
        perm0 = None
        cfg = {"fast": True, "tok0_runs": tok0_runs}
    else:
        # generic fallback: per-row transpose gathers + tok0 gather
        idx1_groups = [
            _flat_idx(d1[:, 0] + i, d1[:, 1] + j, d1[:, 2])
            for i in range(2) for j in range(2)
        ]
        hh, ww = np.arange(2), np.arange(2)
        idx2_groups = [
            _flat_idx(
                (d2[:, 0][None, None, :] + 2 * hh[:, None, None] + i),
                (d2[:, 1][None, None, :] + 2 * ww[None, :, None] + j),
                np.broadcast_to(d2[:, 2][None, None, :], (2, 2, N2)),
            ).ravel()
            for i in range(2) for j in range(2)
        ]
        halves = [np.concatenate(idx1_groups[0:2]), np.concatenate(idx1_groups[2:4]),
                  np.concatenate(idx2_groups[0:2]), np.concatenate(idx2_groups[2:4])]
        parts = [_wrap_idxs(h, 768) for h in halves]
        idx0 = _flat_idx(d0[:, 0], d0[:, 1], d0[:, 2])
        parts.append(_wrap_idxs(idx0, 1408))
        shared["idxs"] = np.concatenate(parts, axis=1)
        cfg = {"fast": False, "tok0_runs": None}

    def _pos(desc, size):
        n = desc.shape[0]
        return np.concatenate(
            [desc[:, :2].astype(np.int32),
             np.full((n, 1), size, np.int32),
             desc[:, 2:3].astype(np.int32)], axis=1)

    positions = np.concatenate(
        [_pos(np.asarray(desc0, np.int32), 1),
         _pos(np.asarray(desc1, np.int32), 2),
         _pos(np.asarray(desc2, np.int32), 4)], axis=0)
    return shared, positions, cfg, perm0, colperm


def kernel(base_patch_embeddings, desc0, desc1, desc2,
           w1, b1, w2a, b2a, w2b, b2b):
    from concourse.bass_utils import run_bass_kernel_spmd

    base = np.asarray(base_patch_embeddings, np.float32)
    assert base.shape == (B, H, W, T, D)

    shared, positions, cfg, perm0, colperm = _prep_shared(
        desc0, desc1, desc2, w1, b1, w2a, b2a, w2b, b2b)

    key = repr(sorted((k, repr(v)) for k, v in cfg.items()))
    if key not in _compiled:
        _compiled[key] = _build_bass(cfg)
    nc = _compiled[key]

    in_maps = []
    for b in range(B):
        sample = np.ascontiguousarray(base[b].reshape(NPOS, D))
        m = dict(shared)
        m["base_f32"] = sample
        m["base_bf16"] = sample.astype(ml_dtypes.bfloat16)
        in_maps.append(m)

    res = run_bass_kernel_spmd(nc, in_maps, core_ids=list(range(N_CORES)))

    tokens = np.empty((B, N0 + N1 + N2, D), np.float32)
    for b in range(B):
        o0 = res.results[b]["out0"]
        oT = res.results[b]["outT"].T                   # [420, 768]
        tokens[b, :N0] = o0[perm0] if perm0 is not None else o0
        tokens[b, N0:] = oT[colperm] if colperm is not None else oT
    return tokens, positions


# revision 72
# speedup vs baseline: 1.4981x; 1.2508x over previous
"""AdaptivePatchEmbed Trainium2 kernel.

Distribution: data-parallel over batch B=8 -> one sample per NeuronCore
(descriptors are identical per sample; small conv weights replicated).

Fast path (used when the descriptors have the 4x4-block structure that
the reference's _build_descs produces; a generic dma_gather path is the
fallback): everything is static HWDGE DMA -- no gpsimd ucode library, no
descriptor generation.
  - scale-0 tokens: ~8 fat static DRAM->DRAM copies in source order; the
    host applies the row permutation during output assembly.
  - conv inputs: the scale-1/scale-2 regions are t-complete 4x4 blocks, so
    they load as ~50 static [<=2 blocks, 16 rows, 1536B] DMAs into
    block-major SBUF tiles [rows, ch]; TensorE transposes them on-chip
    (via identity matmul) into channel-major XT tiles, and the conv
    matmuls read each (i,j)-shifted window directly with a 4-dim strided
    rhs access pattern [block, i', j', t] -- no data permutation at all.
    Output token order becomes (block, i', j', t); the host permutes the
    420 conv tokens back to descriptor order for free during assembly.
  - convs are einsums tok_out[d, n] = sum_{ij,c} W[(ij,c),d] * X[(ij,c),n]
    as 128x128-chunk matmuls accumulating in PSUM; conv2a output stays
    bf16 channel-major in SBUF and feeds conv2b directly.
Outputs per core: out0 [1408,768] f32 (scale-0 rows, source order) and
outT [768,420] f32 (tok1 ++ tok2 transposed, block order).
"""

import numpy as np
import ml_dtypes
from contextlib import ExitStack

# Problem constants (hardcoded; kernel.py must be self-contained).
B, H, W, T, D = 8, 32, 32, 4, 768
N0, N1, N2 = 1408, 336, 84
NPOS = H * W * T              # 4096 rows of D
KC = 24                       # K chunks of 128 over (i,j,c)=4*768
MC = 6                        # output-d chunks of 128
NTOK1 = N1                    # 336 conv1 tokens
NTOK2A = N2 * 4               # 336 conv2a output tokens
NSP = 84                      # spatial tokens per timestep (both convs)
NBLK = 21                     # spatial 4x4 blocks per conv region
NROW = NBLK * 64              # 1344 rows per conv region
NXT = 11                      # 128-row tiles per region (last half-full)
N_CORES = 8

_compiled = {}


def _flat_idx(y, x, t):
    return (y * W + x) * T + t


def _wrap_idxs(idx, pad_to):
    """int sequence -> int16 SBUF wrap layout [128, pad_to//16]."""
    idx = np.asarray(idx, np.int64)
    full = np.zeros(pad_to, np.int64)  # pad with valid idx 0 (junk, never read)
    full[: idx.size] = idx
    assert full.max() < 32768 and pad_to % 16 == 0
    wrapped = full.reshape(pad_to // 16, 16).T.astype(np.int16)  # [16, cols]
    return np.tile(wrapped, (8, 1))  # replicate across the 8 groups of 16


def _weight_mat(w):
    """w [D, D, 2, 2] -> [4D, D] bf16 with wm[(i*2+j)*D + c, d] = w[d,c,i,j].
    k-major layout gives the weight DMA 1536B descriptors, which share the
    SDMA packet round-robin fairly with concurrent SWDGE gather traffic."""
    wm = np.transpose(np.asarray(w, np.float32), (2, 3, 1, 0))  # [i, j, c, d]
    return np.ascontiguousarray(wm.reshape(4 * D, D)).astype(ml_dtypes.bfloat16)


def _bias_tile(b1, b2a, b2b):
    """[128, 18] f32: cols [g*6+m] = b_g[m*128+p]."""
    out = np.empty((128, 18), np.float32)
    for g, b in enumerate((b1, b2a, b2b)):
        out[:, g * 6 : (g + 1) * 6] = np.asarray(b, np.float32).reshape(MC, 128).T
    return out


def _runs_of(blocks):
    """blocks: iterable of (by, bx) 4-aligned. Returns ordered runs
    [(by, bxblk0, stepblk, L)] and the slot order (list of (by,bx))."""
    rows = {}
    for by, bx in blocks:
        rows.setdefault(by, []).append(bx // 4)
    runs, order = [], []
    for by in sorted(rows):
        bxs = sorted(rows[by])
        i = 0
        while i < len(bxs):
            j = i + 1
            step = 1
            if j < len(bxs):
                step = bxs[j] - bxs[i]
                while j + 1 < len(bxs) and bxs[j + 1] - bxs[j] == step:
                    j += 1
            runs.append((by, bxs[i], step, j - i))
            order += [(by, 4 * (bxs[i] + k * step)) for k in range(j - i)]
            i = j
    return runs, order


def _pieces_of(runs):
    """Split runs into <=2-block pieces aligned to even slots.
    Returns [(by, bxblk0, step, L, slot0)]."""
    pieces, c = [], 0
    for by, b0, s, L in runs:
        k = 0
        while k < L:
            if c % 2 == 0 and k + 1 < L:
                pieces.append((by, b0 + k * s, s, 2, c))
                k += 2
                c += 2
            else:
                pieces.append((by, b0 + k * s, s, 1, c))
                k += 1
                c += 1
    return pieces


def _tok0_blocks(desc0):
    """If desc0 is 88 4x4 raster blocks, return [(by, bx, t), ...] else None."""
    d0 = np.asarray(desc0, np.int64)
    if d0.shape != (N0, 3) or N0 % 16:
        return None
    blocks = d0.reshape(N0 // 16, 16, 3)
    by, bx, t = blocks[:, 0, 0], blocks[:, 0, 1], blocks[:, 0, 2]
    yy = by[:, None] + np.repeat(np.arange(4), 4)[None, :]
    xx = bx[:, None] + np.tile(np.arange(4), 4)[None, :]
    tt = np.broadcast_to(t[:, None], yy.shape)
    exp = np.stack([yy, xx, tt], axis=2)
    if not np.array_equal(exp, blocks) or yy.max() >= H or xx.max() >= W:
        return None
    return list(zip(by.tolist(), bx.tolist(), t.tolist()))


def _tok0_runs(tok0_blocks):
    """Returns (runs, pos_of) for the scale-0 fat copies, or (None, None)."""
    if tok0_blocks is None:
        return None, None
    byt = {}
    for by, bx, t in tok0_blocks:
        if by % 4 or bx % 4:
            return None, None
        byt.setdefault((by, bx), set()).add(t)
    if any(ts != set(range(T)) for ts in byt.values()):
        return None, None
    if len(byt) * 16 * T != N0:
        return None, None
    runs, _ = _runs_of(byt.keys())
    raw = []
    for by, b0, s, L in runs:
        for dy in range(4):
            for k in range(L):
                for dx in range(4):
                    for t in range(T):
                        raw.append(((by + dy) * W + (b0 + k * s) * 4 + dx) * T + t)
    raw = np.asarray(raw)
    pos_of = np.full(NPOS, -1, np.int64)
    pos_of[raw] = np.arange(N0)
    return runs, pos_of


def _spatial_lists(d1, d2):
    """Per-timestep spatial token lists (s1_yx [84,2], s2_yx [21,2]) or None."""
    d1 = np.asarray(d1, np.int64)
    d2 = np.asarray(d2, np.int64)
    if d1.shape != (N1, 3) or d2.shape != (N2, 3) or N1 % T or N2 % T:
        return None
    s1 = d1.reshape(T, N1 // T, 3)
    s2 = d2.reshape(T, N2 // T, 3)
    for s, win in ((s1, 2), (s2, 4)):
        if not np.array_equal(s[:, :, 2], np.broadcast_to(
                np.arange(T)[:, None], s.shape[:2])):
            return None
        if not all(np.array_equal(s[0, :, :2], s[t, :, :2]) for t in range(T)):
            return None
        yx = s[0, :, :2]
        if (yx % 2).any() or yx[:, 0].max() + win > H or yx[:, 1].max() + win > W:
            return None
    return s1[0, :, :2], s2[0, :, :2]


def _conv_layout(s1_yx, s2_yx):
    """Block structure for the static conv loads. Returns
    (pieces1, pieces2, perm1, perm2) or None if structure doesn't hold."""
    # s1: group the 84 2x2-token positions into 21 4-aligned 4x4 blocks
    blocks1 = {}
    for s, (y, x) in enumerate(s1_yx):
        by, bx = (y // 4) * 4, (x // 4) * 4
        if y - by not in (0, 2) or x - bx not in (0, 2):
            return None
        blocks1.setdefault((by, bx), {})[((y - by) // 2, (x - bx) // 2)] = s
    if len(blocks1) != NBLK or any(len(v) != 4 for v in blocks1.values()):
        return None
    # s2: the 21 4x4 windows must be 4-aligned blocks
    blocks2 = {}
    for k, (by, bx) in enumerate(s2_yx):
        if by % 4 or bx % 4 or (by, bx) in blocks2:
            return None
        blocks2[(by, bx)] = k
    if len(blocks2) != NBLK:
        return None

    runs1, order1 = _runs_of(blocks1.keys())
    runs2, order2 = _runs_of(blocks2.keys())
    if len(order1) != NBLK or len(order2) != NBLK:
        return None
    slot1 = {b: i for i, b in enumerate(order1)}
    slot2 = {b: i for i, b in enumerate(order2)}

    # conv1 out col for desc1 row (t*84 + s): j'*168 + t*42 + slot*2 + i'
    perm1 = np.empty(N1, np.int64)
    for (by, bx), toks in blocks1.items():
        sl = slot1[(by, bx)]
        for (ip, jp), s in toks.items():
            for t in range(T):
                perm1[t * (N1 // T) + s] = jp * 168 + t * 42 + sl * 2 + ip
    # conv2b out col for desc2 row (t*21 + k): t*21 + slot2
    perm2 = np.empty(N2, np.int64)
    for (by, bx), k in blocks2.items():
        sl = slot2[(by, bx)]
        for t in range(T):
            perm2[t * NBLK + k] = t * NBLK + sl
    return runs1, runs2, perm1, perm2


def _super_idxs(s1_yx, s2_yx):
    """Super-row gather indices (base viewed as [1024, 4*768]): one element
    covers (y, x, all t). (i,j)-major groups. Returns (idx1 [336], idx2 [336])."""
    def srow(y, x):
        return y * W + x

    idx1 = np.concatenate([
        np.asarray([srow(y + i, x + j) for y, x in s1_yx])
        for i in range(2) for j in range(2)])
    # conv2a spatial order (blk, h, w) inside each (i,j)-group
    idx2 = np.concatenate([
        np.asarray([srow(by + 2 * h + i, bx + 2 * w + j)
                    for by, bx in s2_yx for h in range(2) for w in range(2)])
        for i in range(2) for j in range(2)])
    return idx1, idx2


def _build_bass(cfg):
    import concourse.bacc as bacc
    import concourse.tile as tile
    from concourse import mybir

    fast = cfg["fast"]
    nc = bacc.Bacc("TRN2", target_bir_lowering=False, debug=False,
                   num_devices=N_CORES, num_swdge_queues=1,
                   dynamic_dma_scratch_size=24576)
    dt = mybir.dt

    base_f32 = nc.dram_tensor("base_f32", (NPOS, D), dt.float32, kind="ExternalInput")
    base_bf16 = nc.dram_tensor("base_bf16", (NPOS, D), dt.bfloat16, kind="ExternalInput")
    w1m = nc.dram_tensor("w1m", (4 * D, D), dt.bfloat16, kind="ExternalInput")
    w2am = nc.dram_tensor("w2am", (4 * D, D), dt.bfloat16, kind="ExternalInput")
    w2bm = nc.dram_tensor("w2bm", (128, KC * D), dt.bfloat16, kind="ExternalInput")
    biases = nc.dram_tensor("biases", (128, 18), dt.float32, kind="ExternalInput")
    n_idx_cols = 136 if fast else 280
    idxs = nc.dram_tensor("idxs", (128, n_idx_cols), dt.int16, kind="ExternalInput")
    out0 = nc.dram_tensor("out0", (N0, D), dt.float32, kind="ExternalOutput")
    outT = nc.dram_tensor("outT", (D, NTOK1 + N2), dt.float32, kind="ExternalOutput")

    with ExitStack() as ctx:
        tc = ctx.enter_context(tile.TileContext(nc))
        consts = ctx.enter_context(tc.tile_pool(name="consts", bufs=1))
        wpool = ctx.enter_context(tc.tile_pool(name="wpool", bufs=1))
        xpool = ctx.enter_context(tc.tile_pool(name="xpool", bufs=1))
        gpool = ctx.enter_context(tc.tile_pool(name="gpool", bufs=1))
        opool = ctx.enter_context(tc.tile_pool(name="opool", bufs=1))
        psum = ctx.enter_context(tc.tile_pool(name="psum", bufs=4, space="PSUM"))

        # idx upload first on the sync ring: it gates the gathers
        idx_s = consts.tile([128, n_idx_cols], dt.int16)
        nc.sync.dma_start(idx_s[:], idxs.ap()[:])
        bias_s = consts.tile([128, 18], dt.float32)
        nc.scalar.dma_start(bias_s[:], biases.ap()[:])

        if fast:
            # Super-row gathers: 384 idxs of 6.1KB elements (1 x-col x 4t
            # x 768ch contiguous); output [128, 24, 384] with middle dim
            # e = t*6 + c6. Tiny Q7 descriptor-gen (~4us).
            in_v = base_bf16.ap().rearrange("(s a) d -> s (a d)", a=4)
            g1s = gpool.tile([128, 24, 384], dt.bfloat16, tag="g1")
            nc.gpsimd.dma_gather(
                g1s[:], in_v, idx_s[:, 0:24],
                num_idxs=384, num_idxs_reg=384, elem_size=4 * D, transpose=True,
                single_packet=False,
            )
            g2s = gpool.tile([128, 24, 384], dt.bfloat16, tag="g2a")
            nc.gpsimd.dma_gather(
                g2s[:], in_v, idx_s[:, 24:48],
                num_idxs=384, num_idxs_reg=384, elem_size=4 * D, transpose=True,
                single_packet=False,
            )

            def rhs_conv(g, ij, c6, _ntok):
                # cols (t, n): e = t*6 + c6 ; n-group at ij*84
                return g[:, c6 : c6 + 19 : 6, ij * NSP : (ij + 1) * NSP]

            r1key, r2key = g1s, g2s
        else:
            ghalves = []
            for gi in range(4):
                gh = gpool.tile([128, MC, 768], dt.bfloat16, tag=f"gh{gi}")
                nc.gpsimd.dma_gather(
                    gh[:], base_bf16.ap()[:], idx_s[:, gi * 48 : gi * 48 + 48],
                    num_idxs=768, num_idxs_reg=768, elem_size=D, transpose=True,
                    single_packet=False,
                )
                ghalves.append(gh)

            def rhs_conv(gi_base, ij, c6, ntok):
                half = ghalves[gi_base + ij // 2]
                ijl = ij % 2
                return half[:, c6, ijl * ntok : (ijl + 1) * ntok]

            r1key, r2key = 0, 2

        # Weights -> SBUF [128, KC, D] (partition-major DRAM layout).
        # w2b shares w1's slot (loads after conv1 releases it).
        wts = []
        for wdram, nm, tg, eng in (
                (w1m, "w1", "wA", nc.scalar), (w2am, "w2a", "wB", nc.scalar)):
            wt = wpool.tile([128, KC, D], dt.bfloat16, tag=tg, name=nm)
            eng.dma_start(wt[:], wdram.ap().rearrange("(k p) d -> p k d", p=128))
            wts.append(wt)
        w1s, w2as = wts
        w2bs = wpool.tile([128, KC, D], dt.bfloat16, tag="wA", name="w2b")
        nc.scalar.dma_start(w2bs[:], w2bm.ap().rearrange("p (k d) -> p k d", d=D))

        outT_v1 = outT.ap().rearrange("(m p) n -> p m n", p=128)

        # conv1
        out1 = opool.tile([128, MC, NTOK1], dt.float32, tag="out1")
        for m in range(MC):
            ps = psum.tile([128, NTOK1], dt.float32, tag="ps")
            for kc in range(KC):
                ij, c6 = divmod(kc, MC)
                nc.tensor.matmul(
                    ps[:],
                    w1s[:, kc, m * 128 : (m + 1) * 128],
                    rhs_conv(r1key, ij, c6, NTOK1),
                    start=(kc == 0), stop=(kc == KC - 1),
                )
            nc.vector.tensor_scalar_add(out1[:, m, :], ps[:], bias_s[:, m : m + 1])
            nc.sync.dma_start(outT_v1[:, m, 0:NTOK1], out1[:, m, :])

        # conv2a (bf16 output feeds conv2b)
        out2a = opool.tile([128, MC, NTOK2A], dt.bfloat16, tag="out2a")
        for m in range(MC):
            ps = psum.tile([128, NTOK2A], dt.float32, tag="ps")
            for kc in range(KC):
                ij, c6 = divmod(kc, MC)
                nc.tensor.matmul(
                    ps[:],
                    w2as[:, kc, m * 128 : (m + 1) * 128],
                    rhs_conv(r2key, ij, c6, NTOK2A),
                    start=(kc == 0), stop=(kc == KC - 1),
                )
            nc.vector.tensor_scalar_add(out2a[:, m, :], ps[:], bias_s[:, 6 + m : 7 + m])

        # conv2b
        out2b = opool.tile([128, MC, N2], dt.float32, tag="out2b")
        for m in range(MC):
            ps = psum.tile([128, N2], dt.float32, tag="ps")
            for kc in range(KC):
                ij, c6 = divmod(kc, MC)
                if fast:
                    # out2a cols (t, blk, h, w): select (h,w)=(i,j)
                    rhs2b = out2a[:, c6, ij : ij + 4 * (N2 - 1) + 1 : 4]
                else:
                    rhs2b = out2a[:, c6, ij * N2 : (ij + 1) * N2]
                nc.tensor.matmul(
                    ps[:],
                    w2bs[:, kc, m * 128 : (m + 1) * 128],
                    rhs2b,
                    start=(kc == 0), stop=(kc == KC - 1),
                )
            nc.vector.tensor_scalar_add(out2b[:, m, :], ps[:], bias_s[:, 12 + m : 13 + m])
            nc.sync.dma_start(outT_v1[:, m, NTOK1 : NTOK1 + N2], out2b[:, m, :])

        # scale-0: quartered SWDGE row gathers (queued after the conv
        # gathers on the Q7 path) + fat SBUF->DRAM writes; avoids the slow
        # DRAM->DRAM path entirely.
        if cfg["fast"]:
            off_rows, off_cols = 0, 48
            for q, nq in enumerate((384, 384, 384, 256)):
                g0 = gpool.tile([128, 3, D], dt.float32, tag="g0", bufs=2,
                                name=f"g0_{q}")
                gq = nq // 128
                nc.gpsimd.dma_gather(
                    g0[:, 0:gq, :], base_f32.ap()[:],
                    idx_s[:, off_cols : off_cols + nq // 16],
                    num_idxs=nq, num_idxs_reg=nq, elem_size=D,
                    single_packet=False,
                )
                nc.sync.dma_start(
                    out0.ap()[off_rows : off_rows + nq, :]
                    .rearrange("(g p) d -> p g d", p=128),
                    g0[:, 0:gq, :],
                )
                off_rows += nq
                off_cols += nq // 16
        elif cfg["tok0_runs"] is not None:
            base_y = base_f32.ap().rearrange("(y bx rr) d -> y bx rr d",
                                             bx=W // 4, rr=4 * T)
            off = 0
            for k, (by, b0, s, L) in enumerate(cfg["tok0_runs"]):
                nrows = 4 * L * 4 * T
                eng = nc.scalar if k % 2 == 0 else nc.sync
                eng.dma_start(
                    out0.ap()[off : off + nrows, :],
                    base_y[by : by + 4, b0 : b0 + (L - 1) * s + 1 : s, :, :],
                )
                off += nrows
        else:
            g0 = gpool.tile([128, N0 // 128, D], dt.float32, tag="g0")
            nc.gpsimd.dma_gather(
                g0[:], base_f32.ap()[:], idx_s[:, 192:280],
                num_idxs=N0, num_idxs_reg=N0, elem_size=D, single_packet=False,
            )
            nc.sync.dma_start(
                out0.ap().rearrange("(g p) d -> p g d", p=128), g0[:]
            )


    nc.finalize()
    return nc


def _prep_shared(desc0, desc1, desc2, w1, b1, w2a, b2a, w2b, b2b):
    """Host-side shared (core-independent) input prep. Returns (shared,
    positions, cfg, perm0, colperm)."""
    d0 = np.asarray(desc0, np.int64)
    d1 = np.asarray(desc1, np.int64)
    d2 = np.asarray(desc2, np.int64)

    tok0_blocks = _tok0_blocks(d0)
    tok0_runs, pos_of = _tok0_runs(tok0_blocks)
    sp = _spatial_lists(d1, d2)
    fast = sp is not None

    w2bm = _weight_mat(w2b)
    shared = {
        "w1m": _weight_mat(w1),
        "w2am": _weight_mat(w2a),
        # w2b loads late (after conv1 frees its slot) when gather traffic is
        # light, so it uses the fat partition-major layout for ~3x the rate
        "w2bm": np.ascontiguousarray(
            w2bm.reshape(KC, 128, D).transpose(1, 0, 2)).reshape(128, KC * D),
        "biases": _bias_tile(b1, b2a, b2b),
    }

    perm0 = None
    colperm = None
    if fast:
        idx1, idx2 = _super_idxs(*sp)
        idx0 = _flat_idx(d0[:, 0], d0[:, 1], d0[:, 2])
        bounds = [0, 384, 768, 1152, 1408]
        quarters = [_wrap_idxs(idx0[bounds[q] : bounds[q + 1]],
                               bounds[q + 1] - bounds[q]) for q in range(4)]
        shared["idxs"] = np.concatenate(
            [_wrap_idxs(idx1, 384), _wrap_idxs(idx2, 384)] + quarters, axis=1)
        perm0 = None
        cfg = {"fast": True, "tok0_runs": tok0_runs}
    else:
        # generic fallback: per-row transpose gathers + tok0 gather
        idx1_groups = [
            _flat_idx(d1[:, 0] + i, d1[:, 1] + j, d1[:, 2])
            for i in range(2) for j in range(2)
        ]
        hh, ww = np.arange(2), np.arange(2)
        idx2_groups = [
            _flat_idx(
                (d2[:, 0][None, None, :] + 2 * hh[:, None, None] + i),
                (d2[:, 1][None, None, :] + 2 * ww[None, :, None] + j),
                np.broadcast_to(d2[:, 2][None, None, :], (2, 2, N2)),
            ).ravel()
            for i in range(2) for j in range(2)
        ]
        halves = [np.concatenate(idx1_groups[0:2]), np.concatenate(idx1_groups[2:4]),
                  np.concatenate(idx2_groups[0:2]), np.concatenate(idx2_groups[2:4])]
        parts = [_wrap_idxs(h, 768) for h in halves]
        idx0 = _flat_idx(d0[:, 0], d0[:, 1], d0[:, 2])
        parts.append(_wrap_idxs(idx0, 1408))
        shared["idxs"] = np.concatenate(parts, axis=1)
        cfg = {"fast": False, "tok0_runs": None}

    def _pos(desc, size):
        n = desc.shape[0]
        return np.concatenate(
            [desc[:, :2].astype(np.int32),
             np.full((n, 1), size, np.int32),
             desc[:, 2:3].astype(np.int32)], axis=1)

    positions = np.concatenate(
        [_pos(np.asarray(desc0, np.int32), 1),
         _pos(np.asarray(desc1, np.int32), 2),
         _pos(np.asarray(desc2, np.int32), 4)], axis=0)
    return shared, positions, cfg, perm0, colperm


def kernel(base_patch_embeddings, desc0, desc1, desc2,
           w1, b1, w2a, b2a, w2b, b2b):
    from concourse.bass_utils import run_bass_kernel_spmd

    base = np.asarray(base_patch_embeddings, np.float32)
    assert base.shape == (B, H, W, T, D)

    shared, positions, cfg, perm0, colperm = _prep_shared(
        desc0, desc1, desc2, w1, b1, w2a, b2a, w2b, b2b)

    key = repr(sorted((k, repr(v)) for k, v in cfg.items()))
    if key not in _compiled:
        _compiled[key] = _build_bass(cfg)
    nc = _compiled[key]

    in_maps = []
    for b in range(B):
        sample = np.ascontiguousarray(base[b].reshape(NPOS, D))
        m = dict(shared)
        m["base_f32"] = sample
        m["base_bf16"] = sample.astype(ml_dtypes.bfloat16)
        in_maps.append(m)

    res = run_bass_kernel_spmd(nc, in_maps, core_ids=list(range(N_CORES)))

    tokens = np.empty((B, N0 + N1 + N2, D), np.float32)
    for b in range(B):
        o0 = res.results[b]["out0"]
        oT = res.results[b]["outT"].T                   # [420, 768]
        tokens[b, :N0] = o0[perm0] if perm0 is not None else o0
        tokens[b, N0:] = oT[colperm] if colperm is not None else oT
    return tokens, positions
